# revision 11
# baseline (speedup 1.0000x reference)
"""Causal attention (B=4, L=2048, d_model=1024, d_k=d_v=128) on 8 TRN2 NeuronCores.

Sharding (SPMD -- one program, per-core data):
  core c -> batch b = c//2, parity par = c%2.
  Core handles q-blocks j = 2k+par for slot k in 0..7 (128 rows each).
  Each core receives ONLY its own parity's X^T columns (xq).  K/V for the
  other parity are NOT recomputed: each core projects K^T/V for its own
  1024 rows and the pair exchanges them with a 2-replica AllGather through
  DRAM bounce buffers (one for K^T, one for V, staggered so the K wire
  time hides under V's projection).  The readback picks the peer's shard
  with two predicated DMAs (cond = parity of partition_id); the skipped
  one still bumps its semaphore, so the instruction stream stays uniform.

Within a core (all matmuls contract on the partition dim):
  - Q^T/K^T are weight-stationary projections accumulating 8 d_model
    chunks in PSUM; X pieces stream in consumption order.
  - V is projected UN-transposed (X block stationary, W_V moving), so the
    AV matmul's rhs comes straight from the projection -- no PE
    transposes and no extra PSUM round trip.
  - Scores are computed TRANSPOSED: S^T[key, q] = K^T_blk.T @ Q^T.  The
    causal boundary mask is PRELOADED into PSUM (gpsimd copy) and the
    diagonal score matmul accumulates onto it with start=False -- nothing
    sits between the last score matmul and the exp.
  - One merged exp per (parity, key-slot) spanning all covered q-slots
    (up to 1024 wide) writes A^T straight to SBUF bf16.
  - V is augmented with a ones column; Z_aug = A^T.T @ [V | 1] yields the
    softmax denominator in column 128 for free.  Softmax skips the
    row-max subtraction (scores here are bounded ~|12|; exp is safe).
"""

import os
import sys

sys.path.insert(0, "/opt/trn_rl_repo")
sys.path.insert(0, "/opt/trn_rl_repo/concourse")

import ml_dtypes
import numpy as np

import concourse.bass as bass  # noqa: F401
import concourse.mybir as mybir
import concourse.tile as tile
from concourse import bacc
from concourse.bass_utils import run_bass_kernel_spmd
from concourse.masks import make_identity

B, L, DM, DK, DV = 4, 2048, 1024, 128, 128
NB = L // 128   # 16 key blocks per batch
SLOTS = 8       # q-blocks per core
NCH = DM // 128  # 8 d_model chunks
SCALE = float(DK) ** -0.5
MASKVAL = -1e9

DEDUP = os.environ.get("ATTN_DEDUP", "1") == "1"
NWARM = int(os.environ.get("ATTN_NWARM", "14"))

F32 = mybir.dt.float32
BF16 = mybir.dt.bfloat16
VA_W = SLOTS * (DV + 1)  # 1032

# X piece widths (columns of this core's 1024 X^T columns)
PIECES = [128, 128, 256, 512]


def build_nc():
    nc = bacc.Bacc()

    xq_ext = nc.declare_dram_parameter("xq", [DM, SLOTS * 128], BF16, isOutput=False)
    if not DEDUP:
        xo_ext = nc.declare_dram_parameter("xo", [DM, SLOTS * 128], BF16,
                                           isOutput=False)
    # weights pre-arranged on host: [p, c*128+d] = W[c*128+p, d]
    wq_ext = nc.declare_dram_parameter("wq", [128, DM], BF16, isOutput=False)
    wk_ext = nc.declare_dram_parameter("wk", [128, DM], BF16, isOutput=False)
    wv_ext = nc.declare_dram_parameter("wv", [128, DM], BF16, isOutput=False)
    # boundary masks, stored TRANSPOSED for the PE mask-writer matmul:
    # [q 128, 2*128 key] -- col block sp is M_sp^T where M_sp[key, q] is
    # added to the diagonal score block of parity sp (sp=0 own parity:
    # causal triangle; sp=1 other parity: all-masked on even cores /
    # all-zero on odd cores).  The mask enters PSUM via the PE
    # (maskT.T @ I, start=True) so the bank's zero-region marking and the
    # mask write form one deterministic in-order instruction stream --
    # an engine-side PSUM preload would race with the bank's start=True
    # matmul (start lazily zeroes the whole 2KB bank).
    mask_ext = nc.declare_dram_parameter("maskT", [128, 256], BF16, isOutput=False)
    out_ext = nc.declare_dram_parameter("out", [SLOTS * 128, DV], F32, isOutput=True)

    with tile.TileContext(nc) as tc:
        with (
            tc.tile_pool(name="persist", bufs=1) as persist,
            tc.tile_pool(name="st_ps", bufs=2, space="PSUM") as st_ps,
            tc.tile_pool(name="pp_ps", bufs=2, space="PSUM") as pp_ps,
            tc.tile_pool(name="z_ps", bufs=2, space="PSUM") as z_ps,
            tc.tile_pool(name="work", bufs=6) as work,
            tc.tile_pool(name="dram", bufs=4, space="DRAM") as dram,
        ):
            # ---- PE warm-up: ramp DVFS while the first DMAs land ----
            ones = persist.tile([128, 128], BF16, tag="ones")
            nc.vector.memset(ones[:], 1.0)
            warm_ps = z_ps.tile([128, DV + 1], F32, tag="z", name="warm")
            for i in range(NWARM):
                nc.tensor.matmul(warm_ps[:, 0:128], ones[:], ones[:],
                                 start=(i == 0), stop=(i == NWARM - 1))

            # ---- inputs ----
            w_sb = {}

            def load_w(name, ext):
                t = persist.tile([128, NCH, 128], BF16, tag=name, name=name)
                nc.sync.dma_start(
                    out=t[:], in_=ext.rearrange("p (c d) -> p c d", d=128)
                )
                w_sb[name] = t

            xq_r = xq_ext.rearrange("(c p) l -> p c l", p=128)

            def piece(r, lo, w, nm):
                t = persist.tile([128, NCH, w], BF16, tag=nm, name=nm)
                nc.sync.dma_start(out=t[:], in_=r[:, :, lo:lo + w])
                return t

            load_w("wk", wk_ext)
            xq_p = []
            lo = 0
            for i, w in enumerate(PIECES):
                if i == 1:
                    load_w("wv", wv_ext)
                if i == 2:
                    load_w("wq", wq_ext)
                xq_p.append((piece(xq_r, lo, w, f"xq{i}"), lo, w))
                lo += w
            mask_sb = persist.tile([128, 256], BF16, tag="mask")
            nc.sync.dma_start(out=mask_sb[:], in_=mask_ext[:])
            ident = persist.tile([128, 128], BF16, tag="ident")
            make_identity(nc, ident)
            if not DEDUP:
                xo_r = xo_ext.rearrange("(c p) l -> p c l", p=128)
                xo_p = []
                lo = 0
                for i, w in enumerate([512, 512]):
                    xo_p.append((piece(xo_r, lo, w, f"xo{i}"), lo, w))
                    lo += w

            # ---- persistent result tiles ----
            qt = [persist.tile([128, 512], BF16, tag=f"qt{g}", name=f"qt{g}")
                  for g in range(2)]
            # K^T per parity: [d_k 128, 1024 keys]
            kt = [persist.tile([128, SLOTS * 128], BF16, tag=f"kt{sp}",
                               name=f"kt{sp}") for sp in range(2)]
            # V (un-transposed, augmented): per parity one [128, 8*129] tile;
            # block m at cols [m*129, m*129+128], ones at col m*129+128
            va = [persist.tile([128, VA_W], BF16, tag=f"va{sp}", name=f"va{sp}")
                  for sp in range(2)]
            for sp in range(2):
                v3 = va[sp][:].rearrange("p (m x) -> p m x", x=DV + 1)
                nc.vector.memset(v3[:, :, DV:DV + 1], 1.0)
            # A^T tiles: [key 128, q 1024], cols m*128.. used
            at = {(sp, m): persist.tile([128, 1024], BF16, tag=f"at{sp}_{m}",
                                        name=f"at{sp}_{m}")
                  for sp in range(2) for m in range(SLOTS)}

            # ---- projections ----
            def proj_qk(name, dst, pieces, base=0):
                # weight-stationary: dst[128, cols] = W.T @ X (transposed out)
                w = w_sb[name]
                for t, lo, wd in pieces:
                    ps = pp_ps.tile([128, 512], F32, tag="pp", name=f"p{name}")
                    for c in range(NCH):
                        nc.tensor.matmul(
                            ps[:, 0:wd], w[:, c, :], t[:, c, :],
                            start=(c == 0), stop=(c == NCH - 1),
                        )
                    nc.vector.tensor_copy(dst[:, lo - base:lo - base + wd],
                                          ps[:, 0:wd])

            def proj_v(dst_va, pieces):
                # X-block-stationary: V block m = X_blk.T @ W_V  [row, d_v]
                w = w_sb["wv"]
                v3 = dst_va[:].rearrange("p (m x) -> p m x", x=DV + 1)
                for t, lo, wd in pieces:
                    for b0 in range(wd // 128):
                        m = lo // 128 + b0
                        ps = pp_ps.tile([128, 512], F32, tag="pp", name="pv")
                        for c in range(NCH):
                            nc.tensor.matmul(
                                ps[:, 0:DV],
                                t[:, c, b0 * 128:(b0 + 1) * 128],
                                w[:, c, :],
                                start=(c == 0), stop=(c == NCH - 1),
                            )
                        nc.vector.tensor_copy(v3[:, m, 0:DV], ps[:, 0:DV])

            # ---- scores + exp for key-slot m of parity sp ----
            def scores(sp, ms):
                for m in ms:
                    st = st_ps.tile([128, 1024], F32, tag="st", name=f"s{sp}{m}")
                    kcol = kt[sp][:, m * 128:(m + 1) * 128]
                    g0 = m // 4
                    a = m - 4 * g0
                    # diag bank: mask-writer first (start=True zero-marks
                    # the bank and deposits the mask), then the rest and
                    # the diagonal product accumulate in PE issue order.
                    nc.tensor.matmul(
                        st[:, m * 128:(m + 1) * 128],
                        mask_sb[:, sp * 128:(sp + 1) * 128], ident[:],
                        start=True, stop=False, skip_group_check=True,
                    )
                    if a < 3:
                        nc.tensor.matmul(
                            st[:, (m + 1) * 128:(g0 + 1) * 512],
                            kcol, qt[g0][:, (a + 1) * 128:512],
                            start=False, stop=False, skip_group_check=True,
                        )
                    if g0 == 0:
                        nc.tensor.matmul(
                            st[:, 512:1024],
                            kcol, qt[1][:],
                            start=True, stop=True, skip_group_check=True,
                        )
                    nc.tensor.matmul(
                        st[:, m * 128:(m + 1) * 128],
                        kcol, qt[g0][:, a * 128:(a + 1) * 128],
                        start=False, stop=True, skip_group_check=True,
                    )
                    nc.scalar.activation(
                        at[(sp, m)][:, m * 128:1024],
                        st[:, m * 128:1024],
                        mybir.ActivationFunctionType.Exp,
                        bias=0.0, scale=1.0,
                    )

            # ---- A^T.T @ [V|1] for q-slot k ----
            def av(ks):
                for k in ks:
                    zp = z_ps.tile([128, DV + 1], F32, tag="z", name=f"z{k}")
                    for m in range(k + 1):
                        for sp in range(2):
                            nc.tensor.matmul(
                                zp[:],
                                at[(sp, m)][:, k * 128:(k + 1) * 128],
                                va[sp][:, m * (DV + 1):(m + 1) * (DV + 1)],
                                start=(m == 0 and sp == 0),
                                stop=(m == k and sp == 1),
                            )
                    rcp = work.tile([128, 1], F32, tag="rcp")
                    nc.vector.reciprocal(rcp[:], zp[:, DV:DV + 1])
                    z_sb = work.tile([128, DV], F32, tag="zout")
                    nc.vector.tensor_scalar_mul(z_sb[:], zp[:, 0:DV], rcp[:])
                    nc.sync.dma_start(
                        out=out_ext[k * 128:(k + 1) * 128, :], in_=z_sb[:]
                    )

            # ---- emission order (priority under the Tile scheduler) ----
            if DEDUP:
                # project own parity only; exchange K^T and V with the pair
                # core via two staggered AllGathers
                proj_qk("wk", kt[0], xq_p)
                bk_in = dram.tile([128, SLOTS * 128], BF16, name="bk_in")
                bk_out = dram.tile([2, 128, SLOTS * 128], BF16, name="bk_out")
                nc.sync.dma_start(out=bk_in[:], in_=kt[0][:])
                nc.gpsimd.collective_compute(
                    "AllGather",
                    mybir.AluOpType.bypass,
                    replica_groups=[[0, 1], [2, 3], [4, 5], [6, 7]],
                    ins=[bk_in[:]],
                    outs=[bk_out[:]],
                )
                proj_v(va[0], xq_p)
                bv_in = dram.tile([128, SLOTS * 128], BF16, name="bv_in")
                bv_out = dram.tile([2, 128, SLOTS * 128], BF16, name="bv_out")
                v3o = va[0][:].rearrange("p (m x) -> p m x", x=DV + 1)
                nc.sync.dma_start(
                    out=bv_in[:].rearrange("p (m x) -> p m x", x=128),
                    in_=v3o[:, :, 0:DV],
                )
                nc.gpsimd.collective_compute(
                    "AllGather",
                    mybir.AluOpType.bypass,
                    replica_groups=[[0, 1], [2, 3], [4, 5], [6, 7]],
                    ins=[bv_in[:]],
                    outs=[bv_out[:]],
                )
                proj_qk("wq", qt[0], xq_p[0:3])
                proj_qk("wq", qt[1], xq_p[3:4], base=512)

                # predicated readback: even cores (rank 0) read shard 1,
                # odd cores read shard 0; the skipped DMA still increments
                # its semaphore, keeping the stream uniform.
                pid = nc.sync.partition_id()
                par = pid & 1
                v3i = va[1][:].rearrange("p (m x) -> p m x", x=DV + 1)
                for shard in range(2):
                    cond = (par == 1) if shard == 0 else (par == 0)
                    nc.sync.dma_start(out=kt[1][:], in_=bk_out[shard],
                                      cond=cond)
                    nc.sync.dma_start(
                        out=v3i[:, :, 0:DV],
                        in_=bv_out[shard].rearrange("p (m x) -> p m x", x=128),
                        cond=cond,
                    )
                scores(0, range(0, 8))
                for k in range(SLOTS):
                    scores(1, [k])
                    av([k])
            else:
                # fallback: recompute other-parity K/V locally from xo
                proj_qk("wk", kt[0], xq_p)
                proj_v(va[0], xq_p)
                proj_qk("wq", qt[0], xq_p[0:3])
                proj_qk("wq", qt[1], xq_p[3:4], base=512)
                scores(0, range(0, 4))
                proj_qk("wk", kt[1], xo_p)
                proj_v(va[1], xo_p)
                scores(0, range(4, 8))
                for k in range(SLOTS):
                    scores(1, [k])
                    av([k])

    nc.finalize()
    return nc


_NC = None


def _get_nc():
    global _NC
    if _NC is None:
        _NC = build_nc()
    return _NC


def _make_masks():
    p = np.arange(128)[:, None]   # key (partition)
    q = np.arange(128)[None, :]   # query (free)
    triT = np.where(p <= q, 0.0, MASKVAL).astype(np.float32)
    full = np.full((128, 128), MASKVAL, np.float32)
    zero = np.zeros((128, 128), np.float32)
    # col block 0: own-parity key-slot m == k (triangle, both core types);
    # col block 1: other-parity key-slot m == k (all-masked on even cores,
    # all-valid on odd cores).  Stored TRANSPOSED ([q, key]) for the PE
    # mask-writer (lhsT.T @ I puts M back in [key, q] orientation).
    def t(blocks):
        return np.concatenate([b.T for b in blocks], axis=1).astype(
            ml_dtypes.bfloat16)
    mask_even = t([triT, full])
    mask_odd = t([triT, zero])
    return mask_even, mask_odd


def kernel(X, W_Q, W_K, W_V):
    X = np.asarray(X, np.float32)
    W_Q = np.asarray(W_Q, np.float32) * SCALE
    W_K = np.asarray(W_K, np.float32)
    W_V = np.asarray(W_V, np.float32)

    nc = _get_nc()
    mask_even, mask_odd = _make_masks()

    def warr(W):
        return np.ascontiguousarray(
            W.astype(ml_dtypes.bfloat16).reshape(NCH, 128, DK)
            .transpose(1, 0, 2).reshape(128, NCH * DK)
        )

    wq = warr(W_Q)
    wk = warr(W_K)
    wv = warr(W_V)

    in_maps = []
    for c in range(8):
        b, par = c // 2, c % 2
        xt_np = np.ascontiguousarray(X[b].T).astype(ml_dtypes.bfloat16)
        qcols = np.concatenate(
            [np.arange((2 * k + par) * 128, (2 * k + par + 1) * 128)
             for k in range(SLOTS)]
        )
        m = {
            "xq": np.ascontiguousarray(xt_np[:, qcols]),
            "wq": wq, "wk": wk, "wv": wv,
            "maskT": mask_odd if par else mask_even,
        }
        if not DEDUP:
            ocols = np.concatenate(
                [np.arange((2 * k + 1 - par) * 128, (2 * k + 2 - par) * 128)
                 for k in range(SLOTS)]
            )
            m["xo"] = np.ascontiguousarray(xt_np[:, ocols])
        in_maps.append(m)

    res = run_bass_kernel_spmd(nc, in_maps, list(range(8)))

    Z = np.zeros((B, L, DV), np.float32)
    for c in range(8):
        b, par = c // 2, c % 2
        o = res.results[c]["out"]
        for k in range(SLOTS):
            j = 2 * k + par
            Z[b, j * 128:(j + 1) * 128, :] = o[k * 128:(k + 1) * 128, :]
    return Z


# revision 12
# speedup vs baseline: 1.4232x; 1.4232x over previous
"""Causal attention (B=4, L=2048, d_model=1024, d_k=d_v=128) on 8 TRN2 NeuronCores.

Sharding (SPMD -- one program, per-core data):
  core c -> batch b = c//2, parity par = c%2.
  Core handles q-blocks j = 2k+par for slot k in 0..7 (128 rows each).
  Each core receives ONLY its own parity's X^T columns (xq).  K/V for the
  other parity are NOT recomputed: each core projects K^T/V for its own
  1024 rows and the pair exchanges them with a 2-replica AllGather through
  DRAM bounce buffers (one for K^T, one for V, staggered so the K wire
  time hides under V's projection).  The readback picks the peer's shard
  with two predicated DMAs (cond = parity of partition_id); the skipped
  one still bumps its semaphore, so the instruction stream stays uniform.

Within a core (all matmuls contract on the partition dim):
  - Q^T/K^T are weight-stationary projections accumulating 8 d_model
    chunks in PSUM; X pieces stream in consumption order.
  - V is projected UN-transposed (X block stationary, W_V moving), so the
    AV matmul's rhs comes straight from the projection -- no PE
    transposes and no extra PSUM round trip.
  - Scores are computed TRANSPOSED: S^T[key, q] = K^T_blk.T @ Q^T.  The
    causal boundary mask is PRELOADED into PSUM (gpsimd copy) and the
    diagonal score matmul accumulates onto it with start=False -- nothing
    sits between the last score matmul and the exp.
  - One merged exp per (parity, key-slot) spanning all covered q-slots
    (up to 1024 wide) writes A^T straight to SBUF bf16.
  - V is augmented with a ones column; Z_aug = A^T.T @ [V | 1] yields the
    softmax denominator in column 128 for free.  Softmax skips the
    row-max subtraction (scores here are bounded ~|12|; exp is safe).
"""

import os
import sys

sys.path.insert(0, "/opt/trn_rl_repo")
sys.path.insert(0, "/opt/trn_rl_repo/concourse")

import ml_dtypes
import numpy as np

import concourse.bass as bass  # noqa: F401
import concourse.mybir as mybir
import concourse.tile as tile
from concourse import bacc
from concourse.bass_utils import run_bass_kernel_spmd
from concourse.masks import make_identity

B, L, DM, DK, DV = 4, 2048, 1024, 128, 128
NB = L // 128   # 16 key blocks per batch
SLOTS = 8       # q-blocks per core
NCH = DM // 128  # 8 d_model chunks
SCALE = float(DK) ** -0.5
MASKVAL = -1e9

DEDUP = os.environ.get("ATTN_DEDUP", "0") == "1"
NWARM = int(os.environ.get("ATTN_NWARM", "14"))

F32 = mybir.dt.float32
BF16 = mybir.dt.bfloat16
VA_W = SLOTS * (DV + 1)  # 1032

# X piece widths (columns of this core's 1024 X^T columns)
PIECES = [128, 128, 256, 512]


def build_nc():
    nc = bacc.Bacc()

    xq_ext = nc.declare_dram_parameter("xq", [DM, SLOTS * 128], BF16, isOutput=False)
    if not DEDUP:
        xo_ext = nc.declare_dram_parameter("xo", [DM, SLOTS * 128], BF16,
                                           isOutput=False)
    # weights pre-arranged on host: [p, c*128+d] = W[c*128+p, d]
    wq_ext = nc.declare_dram_parameter("wq", [128, DM], BF16, isOutput=False)
    wk_ext = nc.declare_dram_parameter("wk", [128, DM], BF16, isOutput=False)
    wv_ext = nc.declare_dram_parameter("wv", [128, DM], BF16, isOutput=False)
    # boundary masks, stored TRANSPOSED for the PE mask-writer matmul:
    # [q 128, 2*128 key] -- col block sp is M_sp^T where M_sp[key, q] is
    # added to the diagonal score block of parity sp (sp=0 own parity:
    # causal triangle; sp=1 other parity: all-masked on even cores /
    # all-zero on odd cores).  The mask enters PSUM via the PE
    # (maskT.T @ I, start=True) so the bank's zero-region marking and the
    # mask write form one deterministic in-order instruction stream --
    # an engine-side PSUM preload would race with the bank's start=True
    # matmul (start lazily zeroes the whole 2KB bank).
    mask_ext = nc.declare_dram_parameter("maskT", [128, 256], BF16, isOutput=False)
    out_ext = nc.declare_dram_parameter("out", [SLOTS * 128, DV], F32, isOutput=True)

    with tile.TileContext(nc) as tc:
        with (
            tc.tile_pool(name="persist", bufs=1) as persist,
            tc.tile_pool(name="st_ps", bufs=2, space="PSUM") as st_ps,
            tc.tile_pool(name="pp_ps", bufs=2, space="PSUM") as pp_ps,
            tc.tile_pool(name="z_ps", bufs=2, space="PSUM") as z_ps,
            tc.tile_pool(name="work", bufs=6) as work,
            tc.tile_pool(name="dram", bufs=4, space="DRAM") as dram,
        ):
            # ---- PE warm-up: ramp DVFS while the first DMAs land ----
            ones = persist.tile([128, 128], BF16, tag="ones")
            nc.vector.memset(ones[:], 1.0)
            warm_ps = z_ps.tile([128, DV + 1], F32, tag="z", name="warm")
            for i in range(NWARM):
                nc.tensor.matmul(warm_ps[:, 0:128], ones[:], ones[:],
                                 start=(i == 0), stop=(i == NWARM - 1))

            # ---- inputs ----
            w_sb = {}

            def load_w(name, ext):
                t = persist.tile([128, NCH, 128], BF16, tag=name, name=name)
                nc.sync.dma_start(
                    out=t[:], in_=ext.rearrange("p (c d) -> p c d", d=128)
                )
                w_sb[name] = t

            xq_r = xq_ext.rearrange("(c p) l -> p c l", p=128)

            def piece(r, lo, w, nm):
                t = persist.tile([128, NCH, w], BF16, tag=nm, name=nm)
                nc.sync.dma_start(out=t[:], in_=r[:, :, lo:lo + w])
                return t

            load_w("wk", wk_ext)
            xq_p = []
            lo = 0
            for i, w in enumerate(PIECES):
                if i == 1:
                    load_w("wv", wv_ext)
                if i == 2:
                    load_w("wq", wq_ext)
                xq_p.append((piece(xq_r, lo, w, f"xq{i}"), lo, w))
                lo += w
            mask_sb = persist.tile([128, 256], BF16, tag="mask")
            nc.sync.dma_start(out=mask_sb[:], in_=mask_ext[:])
            ident = persist.tile([128, 128], BF16, tag="ident")
            make_identity(nc, ident)
            if not DEDUP:
                xo_r = xo_ext.rearrange("(c p) l -> p c l", p=128)
                xo_p = []
                lo = 0
                for i, w in enumerate([512, 512]):
                    xo_p.append((piece(xo_r, lo, w, f"xo{i}"), lo, w))
                    lo += w

            # ---- persistent result tiles ----
            qt = [persist.tile([128, 512], BF16, tag=f"qt{g}", name=f"qt{g}")
                  for g in range(2)]
            # K^T per parity: [d_k 128, 1024 keys]
            kt = [persist.tile([128, SLOTS * 128], BF16, tag=f"kt{sp}",
                               name=f"kt{sp}") for sp in range(2)]
            # V (un-transposed, augmented): per parity one [128, 8*129] tile;
            # block m at cols [m*129, m*129+128], ones at col m*129+128
            va = [persist.tile([128, VA_W], BF16, tag=f"va{sp}", name=f"va{sp}")
                  for sp in range(2)]
            for sp in range(2):
                v3 = va[sp][:].rearrange("p (m x) -> p m x", x=DV + 1)
                nc.vector.memset(v3[:, :, DV:DV + 1], 1.0)
            # A^T tiles: [key 128, q 1024], cols m*128.. used
            at = {(sp, m): persist.tile([128, 1024], BF16, tag=f"at{sp}_{m}",
                                        name=f"at{sp}_{m}")
                  for sp in range(2) for m in range(SLOTS)}

            # ---- projections ----
            def proj_qk(name, dst, pieces, base=0):
                # weight-stationary: dst[128, cols] = W.T @ X (transposed out)
                w = w_sb[name]
                for t, lo, wd in pieces:
                    ps = pp_ps.tile([128, 512], F32, tag="pp", name=f"p{name}")
                    for c in range(NCH):
                        nc.tensor.matmul(
                            ps[:, 0:wd], w[:, c, :], t[:, c, :],
                            start=(c == 0), stop=(c == NCH - 1),
                        )
                    nc.vector.tensor_copy(dst[:, lo - base:lo - base + wd],
                                          ps[:, 0:wd])

            def proj_v(dst_va, pieces):
                # X-block-stationary: V block m = X_blk.T @ W_V  [row, d_v]
                w = w_sb["wv"]
                v3 = dst_va[:].rearrange("p (m x) -> p m x", x=DV + 1)
                for t, lo, wd in pieces:
                    for b0 in range(wd // 128):
                        m = lo // 128 + b0
                        ps = pp_ps.tile([128, 512], F32, tag="pp", name="pv")
                        for c in range(NCH):
                            nc.tensor.matmul(
                                ps[:, 0:DV],
                                t[:, c, b0 * 128:(b0 + 1) * 128],
                                w[:, c, :],
                                start=(c == 0), stop=(c == NCH - 1),
                            )
                        nc.vector.tensor_copy(v3[:, m, 0:DV], ps[:, 0:DV])

            # ---- scores + exp for key-slot m of parity sp ----
            def scores(sp, ms):
                for m in ms:
                    st = st_ps.tile([128, 1024], F32, tag="st", name=f"s{sp}{m}")
                    kcol = kt[sp][:, m * 128:(m + 1) * 128]
                    g0 = m // 4
                    a = m - 4 * g0
                    # diag bank: mask-writer first (start=True zero-marks
                    # the bank and deposits the mask), then the rest and
                    # the diagonal product accumulate in PE issue order.
                    nc.tensor.matmul(
                        st[:, m * 128:(m + 1) * 128],
                        mask_sb[:, sp * 128:(sp + 1) * 128], ident[:],
                        start=True, stop=False, skip_group_check=True,
                    )
                    if a < 3:
                        nc.tensor.matmul(
                            st[:, (m + 1) * 128:(g0 + 1) * 512],
                            kcol, qt[g0][:, (a + 1) * 128:512],
                            start=False, stop=False, skip_group_check=True,
                        )
                    if g0 == 0:
                        nc.tensor.matmul(
                            st[:, 512:1024],
                            kcol, qt[1][:],
                            start=True, stop=True, skip_group_check=True,
                        )
                    nc.tensor.matmul(
                        st[:, m * 128:(m + 1) * 128],
                        kcol, qt[g0][:, a * 128:(a + 1) * 128],
                        start=False, stop=True, skip_group_check=True,
                    )
                    nc.scalar.activation(
                        at[(sp, m)][:, m * 128:1024],
                        st[:, m * 128:1024],
                        mybir.ActivationFunctionType.Exp,
                        bias=0.0, scale=1.0,
                    )

            # ---- A^T.T @ [V|1] for q-slot k ----
            def av(ks):
                for k in ks:
                    zp = z_ps.tile([128, DV + 1], F32, tag="z", name=f"z{k}")
                    for m in range(k + 1):
                        for sp in range(2):
                            nc.tensor.matmul(
                                zp[:],
                                at[(sp, m)][:, k * 128:(k + 1) * 128],
                                va[sp][:, m * (DV + 1):(m + 1) * (DV + 1)],
                                start=(m == 0 and sp == 0),
                                stop=(m == k and sp == 1),
                            )
                    rcp = work.tile([128, 1], F32, tag="rcp")
                    nc.vector.reciprocal(rcp[:], zp[:, DV:DV + 1])
                    z_sb = work.tile([128, DV], F32, tag="zout")
                    nc.vector.tensor_scalar_mul(z_sb[:], zp[:, 0:DV], rcp[:])
                    nc.sync.dma_start(
                        out=out_ext[k * 128:(k + 1) * 128, :], in_=z_sb[:]
                    )

            # ---- emission order (priority under the Tile scheduler) ----
            if DEDUP:
                # project own parity only; exchange K^T and V with the pair
                # core via two staggered AllGathers
                proj_qk("wk", kt[0], xq_p)
                bk_in = dram.tile([128, SLOTS * 128], BF16, name="bk_in")
                bk_out = dram.tile([2, 128, SLOTS * 128], BF16, name="bk_out")
                nc.sync.dma_start(out=bk_in[:], in_=kt[0][:])
                nc.gpsimd.collective_compute(
                    "AllGather",
                    mybir.AluOpType.bypass,
                    replica_groups=[[0, 1], [2, 3], [4, 5], [6, 7]],
                    ins=[bk_in[:]],
                    outs=[bk_out[:]],
                )
                proj_v(va[0], xq_p)
                bv_in = dram.tile([128, SLOTS * 128], BF16, name="bv_in")
                bv_out = dram.tile([2, 128, SLOTS * 128], BF16, name="bv_out")
                v3o = va[0][:].rearrange("p (m x) -> p m x", x=DV + 1)
                nc.sync.dma_start(
                    out=bv_in[:].rearrange("p (m x) -> p m x", x=128),
                    in_=v3o[:, :, 0:DV],
                )
                nc.gpsimd.collective_compute(
                    "AllGather",
                    mybir.AluOpType.bypass,
                    replica_groups=[[0, 1], [2, 3], [4, 5], [6, 7]],
                    ins=[bv_in[:]],
                    outs=[bv_out[:]],
                )
                proj_qk("wq", qt[0], xq_p[0:3])
                proj_qk("wq", qt[1], xq_p[3:4], base=512)

                # predicated readback: even cores (rank 0) read shard 1,
                # odd cores read shard 0; the skipped DMA still increments
                # its semaphore, keeping the stream uniform.
                pid = nc.sync.partition_id()
                par = pid & 1
                v3i = va[1][:].rearrange("p (m x) -> p m x", x=DV + 1)
                for shard in range(2):
                    cond = (par == 1) if shard == 0 else (par == 0)
                    nc.sync.dma_start(out=kt[1][:], in_=bk_out[shard],
                                      cond=cond)
                    nc.sync.dma_start(
                        out=v3i[:, :, 0:DV],
                        in_=bv_out[shard].rearrange("p (m x) -> p m x", x=128),
                        cond=cond,
                    )
                scores(0, range(0, 8))
                for k in range(SLOTS):
                    scores(1, [k])
                    av([k])
            else:
                # fallback: recompute other-parity K/V locally from xo
                proj_qk("wk", kt[0], xq_p)
                proj_v(va[0], xq_p)
                proj_qk("wq", qt[0], xq_p[0:3])
                proj_qk("wq", qt[1], xq_p[3:4], base=512)
                scores(0, range(0, 4))
                proj_qk("wk", kt[1], xo_p)
                proj_v(va[1], xo_p)
                scores(0, range(4, 8))
                for k in range(SLOTS):
                    scores(1, [k])
                    av([k])

    nc.finalize()
    return nc


_NC = None


def _get_nc():
    global _NC
    if _NC is None:
        _NC = build_nc()
    return _NC


def _make_masks():
    p = np.arange(128)[:, None]   # key (partition)
    q = np.arange(128)[None, :]   # query (free)
    triT = np.where(p <= q, 0.0, MASKVAL).astype(np.float32)
    full = np.full((128, 128), MASKVAL, np.float32)
    zero = np.zeros((128, 128), np.float32)
    # col block 0: own-parity key-slot m == k (triangle, both core types);
    # col block 1: other-parity key-slot m == k (all-masked on even cores,
    # all-valid on odd cores).  Stored TRANSPOSED ([q, key]) for the PE
    # mask-writer (lhsT.T @ I puts M back in [key, q] orientation).
    def t(blocks):
        return np.concatenate([b.T for b in blocks], axis=1).astype(
            ml_dtypes.bfloat16)
    mask_even = t([triT, full])
    mask_odd = t([triT, zero])
    return mask_even, mask_odd


def kernel(X, W_Q, W_K, W_V):
    X = np.asarray(X, np.float32)
    W_Q = np.asarray(W_Q, np.float32) * SCALE
    W_K = np.asarray(W_K, np.float32)
    W_V = np.asarray(W_V, np.float32)

    nc = _get_nc()
    mask_even, mask_odd = _make_masks()

    def warr(W):
        return np.ascontiguousarray(
            W.astype(ml_dtypes.bfloat16).reshape(NCH, 128, DK)
            .transpose(1, 0, 2).reshape(128, NCH * DK)
        )

    wq = warr(W_Q)
    wk = warr(W_K)
    wv = warr(W_V)

    in_maps = []
    for c in range(8):
        b, par = c // 2, c % 2
        xt_np = np.ascontiguousarray(X[b].T).astype(ml_dtypes.bfloat16)
        qcols = np.concatenate(
            [np.arange((2 * k + par) * 128, (2 * k + par + 1) * 128)
             for k in range(SLOTS)]
        )
        m = {
            "xq": np.ascontiguousarray(xt_np[:, qcols]),
            "wq": wq, "wk": wk, "wv": wv,
            "maskT": mask_odd if par else mask_even,
        }
        if not DEDUP:
            ocols = np.concatenate(
                [np.arange((2 * k + 1 - par) * 128, (2 * k + 2 - par) * 128)
                 for k in range(SLOTS)]
            )
            m["xo"] = np.ascontiguousarray(xt_np[:, ocols])
        in_maps.append(m)

    res = run_bass_kernel_spmd(nc, in_maps, list(range(8)))

    Z = np.zeros((B, L, DV), np.float32)
    for c in range(8):
        b, par = c // 2, c % 2
        o = res.results[c]["out"]
        for k in range(SLOTS):
            j = 2 * k + par
            Z[b, j * 128:(j + 1) * 128, :] = o[k * 128:(k + 1) * 128, :]
    return Z
